# revision 4
# baseline (speedup 1.0000x reference)
"""MinGRU (log-space reference) Trainium2 Bass kernel.

Math (normal space, numerically stable since f in (0,1), b >= 0):
    K = x @ Wz + bz ;  T = x @ Wh + bh
    z = sigmoid(K) ;  f = 1 - z ;  g(T) = relu(T) + min(sigmoid(T), 0.5)
    h_t = f_t * h_{t-1} + z_t * g(T_t),  h_init = h0 + 0.5   (h0 in [0,1))
    out = h @ Wo + bo ;  also return h[:, -1:, :]

Sharding: batch (B=8) data-parallel across 8 NeuronCores; weights replicated.
Device layout is fully transposed (features on partitions, sequence on the
free dimension) so the sequential recurrence maps onto tensor_tensor_scan
(one independent fp32 recurrence per partition lane) and no on-device
transposes are needed anywhere.
"""

from contextlib import ExitStack

import numpy as np

import concourse.bacc as bacc
import concourse.bass as bass
import concourse.tile as tile
from concourse import mybir
from concourse.bass_utils import run_bass_kernel_spmd

B, S, E, H = 8, 4096, 1024, 1024
P = 128          # partitions
SC = 512         # sequence chunk (max fp32 moving free dim / one PSUM bank)
NS = S // SC     # 8 sequence chunks
NE = E // P      # 8 input-feature tiles
NH = H // P      # 8 hidden tiles
F32 = mybir.dt.float32
# Matmul operand mode: "f32r" (fp32 data, 4x faster PE path), "f32", "bf16"
MM_MODE = "f32r"
MMDT = {"f32r": mybir.dt.float32r, "f32": F32, "bf16": mybir.dt.bfloat16}[MM_MODE]
import ml_dtypes
MM_NP = {"f32r": np.float32, "f32": np.float32, "bf16": ml_dtypes.bfloat16}[MM_MODE]

_NC_CACHE = {}


def _build_nc():
    nc = bacc.Bacc()

    xt = nc.dram_tensor("xt", [E, S], MMDT, kind="ExternalInput")
    wz = nc.dram_tensor("wz", [E, H], MMDT, kind="ExternalInput")
    wh = nc.dram_tensor("wh", [E, H], MMDT, kind="ExternalInput")
    wo = nc.dram_tensor("wo", [H, E], MMDT, kind="ExternalInput")
    bzt = nc.dram_tensor("bzt", [P, NH], F32, kind="ExternalInput")
    bht = nc.dram_tensor("bht", [P, NH], F32, kind="ExternalInput")
    bot = nc.dram_tensor("bot", [P, NE], F32, kind="ExternalInput")
    ist = nc.dram_tensor("ist", [P, NH], F32, kind="ExternalInput")
    outt = nc.dram_tensor("outt", [E, S], F32, kind="ExternalOutput")
    hlast = nc.dram_tensor("hlast", [P, NH], F32, kind="ExternalOutput")

    with tile.TileContext(nc) as tc, ExitStack() as ctx:
        singles = ctx.enter_context(tc.tile_pool(name="singles", bufs=1))
        xpool = ctx.enter_context(tc.tile_pool(name="xp", bufs=2))
        work = ctx.enter_context(tc.tile_pool(name="work", bufs=2))
        hpool = ctx.enter_context(tc.tile_pool(name="hp", bufs=2))
        cpool = ctx.enter_context(tc.tile_pool(name="cp", bufs=2))
        opool = ctx.enter_context(tc.tile_pool(name="op", bufs=4))
        psum = ctx.enter_context(tc.tile_pool(name="ps", bufs=2, space="PSUM"))
        psum_o = ctx.enter_context(tc.tile_pool(name="pso", bufs=2, space="PSUM"))

        # ---- resident weights / per-partition constants ----
        wz_sb, wh_sb, wo_sb = [], [], []
        for e in range(NE):
            t = singles.tile([P, H], MMDT, tag=f"wz{e}")
            nc.sync.dma_start(out=t[:], in_=wz[e * P:(e + 1) * P, :])
            wz_sb.append(t)
            t = singles.tile([P, H], MMDT, tag=f"wh{e}")
            nc.sync.dma_start(out=t[:], in_=wh[e * P:(e + 1) * P, :])
            wh_sb.append(t)
        for h in range(NH):
            t = singles.tile([P, E], MMDT, tag=f"wo{h}")
            nc.sync.dma_start(out=t[:], in_=wo[h * P:(h + 1) * P, :])
            wo_sb.append(t)
        bzt_sb = singles.tile([P, NH], F32, tag="bzt")
        nc.sync.dma_start(out=bzt_sb[:], in_=bzt[:, :])
        bht_sb = singles.tile([P, NH], F32, tag="bht")
        nc.sync.dma_start(out=bht_sb[:], in_=bht[:, :])
        bot_sb = singles.tile([P, NE], F32, tag="bot")
        nc.sync.dma_start(out=bot_sb[:], in_=bot[:, :])
        ist_sb = singles.tile([P, NH], F32, tag="ist")
        nc.sync.dma_start(out=ist_sb[:], in_=ist[:, :])

        carry = [None] * NH

        def emit_out(j, hs):
            for e in range(NE):
                op = psum_o.tile([P, SC], F32, tag="opsum")
                for h in range(NH):
                    nc.tensor.matmul(
                        op[:],
                        wo_sb[h][:, e * P:(e + 1) * P],
                        hs[h][:],
                        start=(h == 0),
                        stop=(h == NH - 1),
                    )
                ot = opool.tile([P, SC], F32, tag="ot")
                nc.scalar.activation(
                    out=ot[:], in_=op[:],
                    func=mybir.ActivationFunctionType.Identity,
                    bias=bot_sb[:, e:e + 1],
                )
                nc.sync.dma_start(
                    out=outt[e * P:(e + 1) * P, j * SC:(j + 1) * SC], in_=ot[:]
                )

        hs_prev = None
        for j in range(NS):
            xs = []
            for e in range(NE):
                t = xpool.tile([P, SC], MMDT, tag=f"x{e}")
                nc.sync.dma_start(
                    out=t[:], in_=xt[e * P:(e + 1) * P, j * SC:(j + 1) * SC]
                )
                xs.append(t)

            hs = []
            for h in range(NH):
                kp = psum.tile([P, SC], F32, tag="kp")
                tp = psum.tile([P, SC], F32, tag="tp")
                for e in range(NE):
                    nc.tensor.matmul(
                        kp[:],
                        wz_sb[e][:, h * P:(h + 1) * P],
                        xs[e][:],
                        start=(e == 0),
                        stop=(e == NE - 1),
                    )
                for e in range(NE):
                    nc.tensor.matmul(
                        tp[:],
                        wh_sb[e][:, h * P:(h + 1) * P],
                        xs[e][:],
                        start=(e == 0),
                        stop=(e == NE - 1),
                    )
                z = work.tile([P, SC], F32, tag="z")
                nc.scalar.activation(
                    out=z[:], in_=kp[:],
                    func=mybir.ActivationFunctionType.Sigmoid,
                    bias=bzt_sb[:, h:h + 1],
                )
                sg = work.tile([P, SC], F32, tag="sg")
                nc.scalar.activation(
                    out=sg[:], in_=tp[:],
                    func=mybir.ActivationFunctionType.Sigmoid,
                    bias=bht_sb[:, h:h + 1],
                )
                r = work.tile([P, SC], F32, tag="r")
                nc.scalar.activation(
                    out=r[:], in_=tp[:],
                    func=mybir.ActivationFunctionType.Relu,
                    bias=bht_sb[:, h:h + 1],
                )
                # r <- g = min(sg, 0.5) + r
                nc.vector.scalar_tensor_tensor(
                    out=r[:], in0=sg[:], scalar=0.5, in1=r[:],
                    op0=mybir.AluOpType.min, op1=mybir.AluOpType.add,
                )
                # sg <- b = z * g
                nc.vector.tensor_mul(sg[:], z[:], r[:])
                # z <- f = 1 - z
                nc.vector.tensor_scalar(
                    out=z[:], in0=z[:], scalar1=-1.0, scalar2=1.0,
                    op0=mybir.AluOpType.mult, op1=mybir.AluOpType.add,
                )
                ht = hpool.tile([P, SC], MMDT, tag=f"h{h}")
                init = ist_sb[:, h:h + 1] if j == 0 else carry[h][:]
                nc.vector.tensor_tensor_scan(
                    out=ht[:], data0=z[:], data1=sg[:], initial=init,
                    op0=mybir.AluOpType.mult, op1=mybir.AluOpType.add,
                )
                c = cpool.tile([P, 1], F32, tag=f"c{h}")
                nc.gpsimd.tensor_copy(out=c[:], in_=ht[:, SC - 1:SC])
                carry[h] = c
                hs.append(ht)

            if hs_prev is not None:
                emit_out(j - 1, hs_prev)
            hs_prev = hs

        emit_out(NS - 1, hs_prev)

        hl = singles.tile([P, NH], F32, tag="hl")
        for h in range(NH):
            nc.gpsimd.tensor_copy(out=hl[:, h:h + 1], in_=carry[h][:])
        nc.sync.dma_start(out=hlast[:, :], in_=hl[:])

    nc.finalize()
    return nc


def _get_nc():
    if "nc" not in _NC_CACHE:
        _NC_CACHE["nc"] = _build_nc()
    return _NC_CACHE["nc"]


def _tilecols(v, n):
    # (n*P,) -> (P, n) with [:, i] = v[i*P:(i+1)*P]
    return np.ascontiguousarray(v.reshape(n, P).T.astype(np.float32))


def _prepare_in_maps(x, h_0, Wz, bz, Wh, bh, Wo, bo):
    x = np.asarray(x, dtype=np.float32)
    h_0 = np.asarray(h_0, dtype=np.float32)
    shared = {
        "wz": np.ascontiguousarray(np.asarray(Wz).astype(MM_NP)),
        "wh": np.ascontiguousarray(np.asarray(Wh).astype(MM_NP)),
        "wo": np.ascontiguousarray(np.asarray(Wo).astype(MM_NP)),
        "bzt": _tilecols(np.asarray(bz, dtype=np.float32), NH),
        "bht": _tilecols(np.asarray(bh, dtype=np.float32), NH),
        "bot": _tilecols(np.asarray(bo, dtype=np.float32), NE),
    }
    in_maps = []
    for b in range(B):
        m = dict(shared)
        m["xt"] = np.ascontiguousarray(x[b].T.astype(MM_NP))  # (E, S)
        m["ist"] = _tilecols(h_0[b, 0, :] + np.float32(0.5), NH)
        in_maps.append(m)
    return in_maps


def _postprocess(results):
    out = np.empty((B, S, E), dtype=np.float32)
    h_last = np.empty((B, 1, H), dtype=np.float32)
    for b in range(B):
        out[b] = results[b]["outt"].T
        h_last[b, 0, :] = results[b]["hlast"].T.reshape(H)
    return out, h_last


def run(trace=False, **inputs):
    in_maps = _prepare_in_maps(**inputs)
    nc = _get_nc()
    res = run_bass_kernel_spmd(nc, in_maps, core_ids=list(range(B)), trace=trace)
    out, h_last = _postprocess(res.results)
    return (out, h_last), res


def kernel(**inputs):
    outputs, _ = run(trace=False, **inputs)
    return outputs
